# revision 29
# baseline (speedup 1.0000x reference)
"""Always-on MoE forward (expert 0 dense + top-k of 7 routed) on 8 TRN2 cores.

Strategy
--------
The router (4096x1024 @ 1024x7 matmul + softmax + top-2) is ~58 MFLOP --
negligible -- so it runs on host in numpy as part of computing the sharding
plan.  The expensive part (expert SwiGLU MLPs, ~155 GFLOP with top-2
sparsity) runs on device, expert-parallel with host-side token
dispatch/combine:

- SPMD graph: every core runs TWO or THREE weight groups with compile-time
  capacities.  Each (core, group) slot is filled with tokens of ONE expert
  (weights supplied per-core via in_maps).  A capacity search (2-group
  legacy + 3-group DFS assignment where an expert may span several slots on
  different cores) picks the group sizes minimizing the modeled PE-stream
  time; with 3 groups the per-core capacity lands within ~1% of the ideal
  (T + k*T) / 8 balance point (e.g. caps (344, 412, 792) -> 1548 vs 1536
  ideal, vs 1586 for the best 2-group plan).  Leftover slots are filled
  with always-on expert-0 tokens.
- Host gathers each core's tokens (transposed k-chunk layout, bf16), device
  computes down(silu(x@wg) * (x@wu)) for both groups, host scatter-adds the
  outputs with the combine weights (expert-0 weight 1.0).

Device kernel: pure dense matmul streaming, weights stationary in SBUF per
group.  All DRAM views are pre-tiled on host into [128, flat] layouts so
every DMA is contiguous per partition and every matmul operand is a direct
SBUF slice; weights are DMA'd per 128-wide m-tile (0.25 MB) so the first
matmuls start ~8 us into the kernel instead of waiting for full weight sets.
A 38-matmul PE pre-warm burst accumulates the ~3.4 us of sustained activity
the HAM clock gate needs, so the gate flips to 8/8 during the DMA ramp, and
the final tile's last m2 output is split column-wise so its cast+DMA overlap
the closing matmuls (shorter serial tail).
"""

import numpy as np
import ml_dtypes

D = 1024
DFF = 2048
E = 8
NCORES = 8
T = 2 * 2048  # B * S
KD = D // 128    # contraction chunks over D
KF = DFF // 128  # contraction chunks over DFF

_COMPILED = {}

_BF16 = ml_dtypes.bfloat16

LAST_EXEC_NS = None
LAST_RESULT = None


def _route(x, router_w, router_b, top_k):
    """Replicates the reference router in numpy f32: returns (topi, topw)."""
    logits = x.astype(np.float32) @ router_w.astype(np.float32) + router_b.astype(
        np.float32
    )
    m = logits.max(axis=-1, keepdims=True)
    p = np.exp(logits - m)
    p /= p.sum(axis=-1, keepdims=True)
    k = int(top_k)
    topi = np.argpartition(-p, kth=k - 1, axis=-1)[:, :k]  # top-k set (unordered)
    topw = np.take_along_axis(p, topi, axis=-1)
    topw = topw / topw.sum(axis=-1, keepdims=True)
    return topi, topw.astype(np.float32)


def _split_even(idx, w, n):
    """Split (idx, w) into n near-equal chunks."""
    c = len(idx)
    sizes = [(c + n - 1 - i) // n for i in range(n)]
    out, pos = [], 0
    for s in sizes:
        out.append((idx[pos : pos + s], w[pos : pos + s]))
        pos += s
    return out


def _widths_for(cap):
    """Tile widths for a group capacity.

    First tile as wide as possible (512) -- during the first tile the weight
    stream runs near the HBM rate, and a wider tile lowers the per-ns weight
    demand.  Remaining capacity in near-equal tiles, kept above the ~280
    matmul instruction floor (LDWEIGHTS + dispatch) when possible.
    """
    if cap <= 0:
        return []

    def near_equal(c, n):
        return [(c + n - 1 - i) // n for i in range(n)]

    plain = near_equal(cap, -(-cap // 512))
    if cap > 512:
        rem = cap - 512
        lead = [512] + near_equal(rem, -(-rem // 512))
    else:
        lead = plain
    cost = lambda ws: sum(max(w, 280) for w in ws)
    return lead if cost(lead) <= cost(plain) else plain


def _tile_cost(cap):
    """Modeled PE-stream ns for one group: 384 matmuls per tile, each
    max(streaming, LDWEIGHTS-floor ~108ns)."""
    return sum(384 * max(w / 2.4 + 2.5, 108.0) for w in _widths_for(cap))


def _assign_k3(caps, counts_list):
    """DFS: assign each routed expert a (i0,i1,i2) slot combo (i_g slots of
    group g); e0 must fit in the leftover slots.  Cost is fixed by `caps`,
    so the FIRST feasible assignment wins (min-capacity-first ordering keeps
    leftover for e0 large).  Returns {expert: (i0,i1,i2)} or None."""
    sizes = caps
    total_cap = NCORES * sum(sizes)
    order = sorted(counts_list, key=lambda ec: -ec[1])
    combos_per = []
    for e, c in order:
        opts = []
        for i0 in range(4):
            for i1 in range(4):
                for i2 in range(4):
                    capx = i0 * sizes[0] + i1 * sizes[1] + i2 * sizes[2]
                    if capx >= c:
                        opts.append((capx, (i0, i1, i2)))
        opts.sort()
        if not opts:
            return None
        combos_per.append((e, c, opts[:10]))
    # lower bound on remaining assigned capacity from expert idx onward
    lb = [0] * (len(combos_per) + 1)
    for i in range(len(combos_per) - 1, -1, -1):
        lb[i] = lb[i + 1] + combos_per[i][2][0][0]
    found = [None]
    budget = [60000]
    def dfs(idx, use, tot, picks):
        if found[0] is not None or budget[0] <= 0:
            return
        budget[0] -= 1
        if total_cap - tot - lb[idx] < T:
            return  # e0 can no longer fit
        if idx == len(combos_per):
            found[0] = dict(picks)
            return
        e, c, opts = combos_per[idx]
        for capx, inc in opts:
            nu = [u + i for u, i in zip(use, inc)]
            if any(u > NCORES for u in nu):
                continue
            picks.append((e, inc))
            dfs(idx + 1, nu, tot + capx, picks)
            picks.pop()
            if found[0] is not None:
                return
    dfs(0, [0, 0, 0], 0, [])
    return found[0]


def _assign_kn(caps, counts_list, budget=30000):
    """Generic n-group DFS assignment (see _assign_k3)."""
    import itertools

    n = len(caps)
    total_cap = NCORES * sum(caps)
    order = sorted(counts_list, key=lambda ec: -ec[1])
    combos_per = []
    for e, c in order:
        opts = []
        for inc in itertools.product(range(4), repeat=n):
            if sum(inc) > 4:
                continue
            capx = sum(i * s for i, s in zip(inc, caps))
            if capx >= c:
                opts.append((capx, inc))
        opts.sort()
        if not opts:
            return None
        combos_per.append((e, c, opts[:8]))
    lb = [0] * (len(combos_per) + 1)
    for i in range(len(combos_per) - 1, -1, -1):
        lb[i] = lb[i + 1] + combos_per[i][2][0][0]
    found = [None]
    bud = [budget]

    def dfs(idx, use, tot, picks):
        if found[0] is not None or bud[0] <= 0:
            return
        bud[0] -= 1
        if total_cap - tot - lb[idx] < T:
            return
        if idx == len(combos_per):
            found[0] = dict(picks)
            return
        e, c, opts = combos_per[idx]
        for capx, inc in opts:
            nu = [u + i for u, i in zip(use, inc)]
            if any(u > NCORES for u in nu):
                continue
            picks.append((e, inc))
            dfs(idx + 1, nu, tot + capx, picks)
            picks.pop()
            if found[0] is not None:
                return
    dfs(0, [0] * n, 0, [])
    return found[0]


def _plan_k4(counts):
    """Search 4-group capacities (all single tiles <= 512): finer packing
    than 3 groups cuts the padded columns (e.g. 1548 -> 1540 = -1.3 us of
    PE stream).  Among equal-sum partitions, prefer the WIDEST head tile
    (the first group's tile width sets the weight-demand rate during the
    HBM-contended ramp, so closer to 512 is safer)."""
    counts_list = [(e, c) for e, c in counts.items() if c > 0]
    lo = -(-(T + sum(counts.values())) // NCORES)
    s0 = lo + (-lo) % 4
    for S in range(s0, s0 + 17, 4):
        cands = []
        for A in range(256, 513, 4):
            for B in range(A, 513, 4):
                for C in range(B, 513, 4):
                    Dd = S - A - B - C
                    if Dd < C or Dd > 512:
                        continue
                    cands.append((A, B, C, Dd))
        cands.sort(key=lambda t: (-t[3], t[0]))
        for caps in cands:
            picks = _assign_kn(list(caps), counts_list)
            if picks is not None:
                return sum(_tile_cost(c) for c in caps), tuple(caps), picks
    return None


def _plan_k2(counts):
    """Legacy 2-group plan: (cost, caps, combos) -- each routed expert gets
    j B-slots; e0 fills the rest."""
    cands = sorted(
        {-(-c // j) for c in counts.values() for j in range(1, 9) if c} | {512}
    )
    best = None
    for Bc in cands:
        npieces = sum(-(-c // Bc) for c in counts.values() if c)
        if npieces > NCORES:
            continue
        nfree = NCORES - npieces
        e0_in_b = min(T, nfree * Bc)
        A = -(-(T - e0_in_b) // NCORES) if e0_in_b < T else 0
        tot = _tile_cost(A) + _tile_cost(Bc)
        if best is None or tot < best[0]:
            best = (tot, A, Bc)
    tot, A, B = best
    combos = {e: (0, -(-c // B)) for e, c in counts.items() if c}
    return tot, (A, B), combos


def _plan_k3(counts):
    """Search 3-group capacities; returns (cost, caps, combos) or None."""
    from functools import lru_cache

    tc = lru_cache(maxsize=None)(_tile_cost)
    lo = -(-(T + sum(counts.values())) // NCORES)
    counts_list = [(e, c) for e, c in counts.items() if c > 0]
    cands = []
    for C in range(560, 1241, 4):
        cC = tc(C)
        for B in range(240, C + 1, 4):
            cB = tc(B)
            a0 = max(160, lo - B - C)
            a1 = min(B, lo + 52 - B - C)
            a0 += (-a0) % 4
            for A in range(a0, a1 + 1, 4):
                cands.append((tc(A) + cB + cC, A, B, C))
    cands.sort()
    tried = 0
    for cost, A, B, C in cands:
        tried += 1
        if tried > 25000:
            break
        picks = _assign_k3((A, B, C), counts_list)
        if picks is not None:
            return cost, (A, B, C), picks
    return None


def _plan_slots(per_expert):
    """Pick group capacities and fill the 8 slots per group.

    per_expert: {e: (idx, w)} for routed experts.
    Returns (caps, slots); slots[c] = tuple over groups of (e, idx, w).
    """
    counts = {e: len(v[0]) for e, v in per_expert.items()}
    cost2, caps2, combos2 = _plan_k2(counts)
    plan3 = _plan_k3(counts)
    if plan3 is not None and plan3[0] < cost2 - 2000:
        cost, caps, combos = plan3
    else:
        cost, caps, combos = cost2, caps2, combos2
    plan4 = _plan_k4(counts)
    if plan4 is not None and plan4[0] < cost - 600:
        cost, caps, combos = plan4

    ngroups = len(caps)
    ones = np.ones(T, dtype=np.float32)
    e0_idx = np.arange(T, dtype=np.int64)

    # Build expert pieces per group: fill assigned slots, remainder last.
    group_pieces = [[] for _ in range(ngroups)]
    for e in sorted(counts, key=lambda e: -counts[e]):
        if counts[e] == 0:
            continue
        idx, w = per_expert[e]
        combo = combos[e]
        slot_sizes = []
        for g in range(ngroups):
            slot_sizes += [(g, caps[g])] * combo[g]
        # fill largest slots first so at most one slot is partial
        slot_sizes.sort(key=lambda gs: -gs[1])
        pos = 0
        for g, size in slot_sizes:
            take = min(size, len(idx) - pos)
            group_pieces[g].append((e, idx[pos : pos + take], w[pos : pos + take]))
            pos += take
        assert pos >= len(idx), f"expert {e} tokens not covered"
    # e0 fills remaining slots (largest first so at most one is partial)
    free = []
    for g in range(ngroups):
        free += [(g, caps[g])] * (NCORES - len(group_pieces[g]))
    free.sort(key=lambda gs: -gs[1])
    pos = 0
    for g, size in free:
        take = min(size, T - pos)
        group_pieces[g].append((0, e0_idx[pos : pos + take], ones[pos : pos + take]))
        pos += take
    assert pos >= T, "expert-0 tokens not covered"

    slots = [
        tuple(group_pieces[g][c] for g in range(ngroups)) for c in range(NCORES)
    ]
    return caps, slots


def _wgu_layout(w2d):
    """[D, DFF] f32 -> [128, KF*KD*128] bf16 m-tile-major layout:
    element [p, (m*KD + k)*128 + c] = W[k*128 + p, m*128 + c]."""
    a = w2d.reshape(KD, 128, KF, 128).transpose(1, 2, 0, 3).reshape(128, -1)
    return np.ascontiguousarray(a).astype(_BF16)


def _wd_layout(w2d):
    """[DFF, D] f32 -> [128, KD*KF*128] bf16 m-tile-major layout:
    element [p, (m*KF + k)*128 + c] = W[k*128 + p, m*128 + c]."""
    a = w2d.reshape(KF, 128, KD, 128).transpose(1, 2, 0, 3).reshape(128, -1)
    return np.ascontiguousarray(a).astype(_BF16)


def _tiles_for(caps):
    """Token tiles [(group, start_in_group, width, flat_offset)] per core."""
    tiles = []
    off = 0
    for g, cap in enumerate(caps):
        ts = 0
        for w in _widths_for(cap):
            tiles.append((g, ts, w, off))
            off += w
            ts += w
    return tiles, off


def _build_graph(caps):
    import concourse.mybir as mybir
    import concourse.tile as tile
    from concourse import bacc
    from contextlib import ExitStack

    bf16 = mybir.dt.bfloat16
    f32 = mybir.dt.float32

    tiles, total = _tiles_for(caps)
    groups = [g for g in range(len(caps)) if caps[g] > 0]

    nc = bacc.Bacc("TRN2", target_bir_lowering=False)

    xt_d = nc.declare_dram_parameter("xt", [128, KD * total], bf16, isOutput=False)
    w_ds = []
    for g in groups:
        w_ds.append(
            (
                nc.declare_dram_parameter(
                    f"w{g}g", [128, KF * KD * 128], bf16, isOutput=False
                ),
                nc.declare_dram_parameter(
                    f"w{g}u", [128, KF * KD * 128], bf16, isOutput=False
                ),
                nc.declare_dram_parameter(
                    f"w{g}d", [128, KD * KF * 128], bf16, isOutput=False
                ),
            )
        )
    out_d = nc.declare_dram_parameter("out", [128, KD * total], bf16, isOutput=True)

    with tile.TileContext(nc) as tc, ExitStack() as ctx:
        # One SBUF pool + one PSUM pool (per-tag bufs) -- the end-of-kernel
        # semaphore drain scales with pool count, so fewer pools shorten the
        # epilogue.
        sbp = ctx.enter_context(tc.tile_pool(name="sb", bufs=1))
        wpool = xpool = hpool = gpool = opool = sbp
        psp = ctx.enter_context(tc.tile_pool(name="ps", bufs=2, space="PSUM"))
        psg = psu = psd = psp

        GU_M = KD * 128  # bytes-per-m-tile span (elems) for wg/wu
        D_M = KF * 128   # for wd

        # PE pre-warm: a short burst of dummy matmuls bridges the ~2us gap
        # until the first weight/token chunks land, and starts the HAM
        # busy-window early so the clock gate flips to 8/8 sooner.  Real
        # matmuls start as soon as DMA delivers (cold at first, but doing
        # real work at 1.2 GHz beats idling warm).
        warm_sb = wpool.tile([128, 128], bf16, tag="warm")
        nc.gpsimd.memset(warm_sb[:], 0)
        ps_w = psp.tile([128, 128], f32, tag="psw", bufs=1)
        for _ in range(38):
            nc.tensor.matmul(ps_w[:], warm_sb[:], warm_sb[:], start=True, stop=True)

        # Larger group first: its weights stream in unblocked at t=0, and the
        # other group's weight reloads (WAR-gated on this group's last use of
        # each m-tile slot) get a long compute window to hide under.
        order = sorted(range(len(groups)), key=lambda gi: -caps[groups[gi]])

        # Weight DMA blocks: the HWDGE issue path allows only ~6 outstanding
        # dma_starts per engine (each further issue waits a prior completion
        # sem), so many small DMAs serialize on sem round-trips.  Geometric
        # doubling gives a tiny first block (first matmul starts early) and
        # few total DMAs.  wg+wd ride sync (wd queued behind wg keeps it off
        # the ramp-critical bytes; SDMA bandwidth is shared, so concurrent
        # wd DMAs starve the wg/wu/xt lead blocks -- measured +23 us), and
        # xt + wu + outs ride scalar, with xt tile0 split into 4 k-chunks
        # interleaved with the wu lead blocks so the first real matmul
        # starts ~0.4 MB into the stream instead of 1.25 MB.
        GU_BLKS = [(0, 1), (1, 2), (2, 4), (4, 8), (8, 16)]   # m-tile ranges
        WD_BLKS = [(0, 2), (2, 4), (4, 8)]                    # m2-tile ranges

        for gi in order:
            g = groups[gi]
            wg_d, wu_d, wd_d = w_ds[gi]
            first_group = gi == order[0]
            wg_blk, wu_blk, wd_blk = [], [], []
            for bi, (s, e) in enumerate(GU_BLKS):
                span = (e - s) * GU_M
                tg = wpool.tile([128, span], bf16, tag=f"wg_b{bi}")
                nc.sync.dma_start(tg[:], wg_d.ap()[:, s * GU_M : e * GU_M])
                wg_blk.append(tg)

            xt0_sb = None
            if first_group:
                t0_off, t0_w = next(
                    (off, w_) for tg2, ts2, w_, off in tiles if tg2 == g
                )
                xt0_sb = xpool.tile([128, KD * t0_w], bf16, tag="xt", bufs=3)

                def xt0_chunk(ci, t=xt0_sb, off=t0_off, w_=t0_w):
                    # chunks 0-1 ride scalar HWDGE (fast first byte); 2-3
                    # ride the otherwise-idle gpsimd SWDGE (~2us latency,
                    # fine for data needed ~5-8us in).
                    eng = nc.scalar if ci < 2 else nc.gpsimd
                    ks, ke = 2 * ci, 2 * ci + 2
                    eng.dma_start(
                        t[:, ks * w_ : ke * w_],
                        xt_d.ap()[:, KD * off + ks * w_ : KD * off + ke * w_],
                    )

                xt0_chunk(0)
                xt0_chunk(2)
                xt0_chunk(3)
            for bi, (s, e) in enumerate(GU_BLKS):
                span = (e - s) * GU_M
                tu = wpool.tile([128, span], bf16, tag=f"wu_b{bi}")
                nc.scalar.dma_start(tu[:], wu_d.ap()[:, s * GU_M : e * GU_M])
                wu_blk.append(tu)
                if first_group and bi == 0:
                    xt0_chunk(1)
            for bi, (s, e) in enumerate(WD_BLKS):
                span = (e - s) * D_M
                td = wpool.tile([128, span], bf16, tag=f"wd_b{bi}")
                nc.sync.dma_start(td[:], wd_d.ap()[:, s * D_M : e * D_M])
                wd_blk.append(td)

            def gu_slice(blk_list, m, k):
                for bi, (s, e) in enumerate(GU_BLKS):
                    if s <= m < e:
                        base = ((m - s) * KD + k) * 128
                        return blk_list[bi][:, base : base + 128]
                raise AssertionError

            def wd_slice(m2, k2):
                for bi, (s, e) in enumerate(WD_BLKS):
                    if s <= m2 < e:
                        base = ((m2 - s) * KF + k2) * 128
                        return wd_blk[bi][:, base : base + 128]
                raise AssertionError

            group_tiles = [t for t in tiles if t[0] == g]
            for ti, (tg_, ts, w, off) in enumerate(group_tiles):
                final_tile = gi == order[-1] and ti == len(group_tiles) - 1
                if first_group and ts == 0:
                    rhs = lambda k, t=xt0_sb: t[:, k * w : k * w + w]
                else:
                    xt_sb = xpool.tile([128, KD * w], bf16, tag="xt", bufs=3)
                    nc.scalar.dma_start(
                        xt_sb[:], xt_d.ap()[:, KD * off : KD * (off + w)]
                    )
                    rhs = lambda k, t=xt_sb: t[:, k * w : k * w + w]
                h_sb = hpool.tile([128, KF, 512], bf16, tag="h", bufs=2)
                for m in range(KF):
                    ps_g = psg.tile([128, 512], f32, tag="psg")
                    ps_u = psu.tile([128, 512], f32, tag="psu")
                    for k in range(KD):
                        nc.tensor.matmul(
                            ps_g[:, :w],
                            gu_slice(wg_blk, m, k),
                            rhs(k),
                            start=(k == 0),
                            stop=(k == KD - 1),
                        )
                    for k in range(KD):
                        nc.tensor.matmul(
                            ps_u[:, :w],
                            gu_slice(wu_blk, m, k),
                            rhs(k),
                            start=(k == 0),
                            stop=(k == KD - 1),
                        )
                    g_sb = gpool.tile([128, 512], bf16, tag="gact", bufs=3)
                    nc.scalar.activation(
                        g_sb[:, :w],
                        ps_g[:, :w],
                        mybir.ActivationFunctionType.Silu,
                    )
                    nc.vector.tensor_mul(h_sb[:, m, :w], g_sb[:, :w], ps_u[:, :w])
                # Output: one DMA per tile (fewer descriptors, shorter
                # end-of-kernel drain), except the final tile which keeps
                # per-m2 DMAs so the last bytes leave as soon as computed.
                o_sb = opool.tile([128, KD * w], bf16, tag="o", bufs=2)
                for m2 in range(KD):
                    if final_tile and m2 == KD - 1:
                        # Split the last m2 column-wise and alternate the
                        # out DMAs across the Act and (idle) sync rings:
                        # the cast+DMA of the first half overlaps the
                        # second half's matmuls, and the very last DMA
                        # issues onto an empty queue.
                        hw1 = (w // 2 + 3) & ~3
                        for ci2, (cs, cw) in enumerate(((0, hw1), (hw1, w - hw1))):
                            ps_d = psd.tile([128, 512], f32, tag="psd")
                            for k2 in range(KF):
                                nc.tensor.matmul(
                                    ps_d[:, :cw],
                                    wd_slice(m2, k2),
                                    h_sb[:, k2, cs : cs + cw],
                                    start=(k2 == 0),
                                    stop=(k2 == KF - 1),
                                )
                            nc.vector.tensor_copy(
                                o_sb[:, m2 * w + cs : m2 * w + cs + cw],
                                ps_d[:, :cw],
                            )
                            eng = nc.scalar if ci2 == 0 else nc.sync
                            eng.dma_start(
                                out_d.ap()[
                                    :,
                                    KD * off + m2 * w + cs : KD * off
                                    + m2 * w
                                    + cs
                                    + cw,
                                ],
                                o_sb[:, m2 * w + cs : m2 * w + cs + cw],
                            )
                        continue
                    ps_d = psd.tile([128, 512], f32, tag="psd")
                    for k2 in range(KF):
                        nc.tensor.matmul(
                            ps_d[:, :w],
                            wd_slice(m2, k2),
                            h_sb[:, k2, :w],
                            start=(k2 == 0),
                            stop=(k2 == KF - 1),
                        )
                    nc.vector.tensor_copy(o_sb[:, m2 * w : (m2 + 1) * w], ps_d[:, :w])
                    if final_tile and m2 == KD - 3:
                        # one batched DMA for m2 0-5 (fewer Act-queue issue
                        # slices ahead of the tail-critical DMAs)
                        nc.scalar.dma_start(
                            out_d.ap()[:, KD * off : KD * off + (KD - 2) * w],
                            o_sb[:, 0 : (KD - 2) * w],
                        )
                    elif final_tile and m2 == KD - 2:
                        nc.sync.dma_start(
                            out_d.ap()[
                                :, KD * off + m2 * w : KD * off + (m2 + 1) * w
                            ],
                            o_sb[:, m2 * w : (m2 + 1) * w],
                        )
                if not final_tile:
                    nc.scalar.dma_start(
                        out_d.ap()[:, KD * off : KD * (off + w)], o_sb[:]
                    )

    nc.compile()
    return nc


def kernel(hidden_states, router_w, router_b, wg, wu, wd, top_k):
    hidden_states = np.asarray(hidden_states, dtype=np.float32)
    router_w = np.asarray(router_w, dtype=np.float32)
    router_b = np.asarray(router_b, dtype=np.float32)
    wg = np.asarray(wg, dtype=np.float32)
    wu = np.asarray(wu, dtype=np.float32)
    wd = np.asarray(wd, dtype=np.float32)

    Bb, S, Dd = hidden_states.shape
    x = hidden_states.reshape(-1, Dd)
    assert x.shape == (T, D)

    topi, topw = _route(x, router_w, router_b, top_k)
    per_expert = {}
    for e in range(1, E):
        sel = np.nonzero((topi == (e - 1)).any(axis=1))[0]
        w = topw[sel][topi[sel] == (e - 1)]
        per_expert[e] = (sel.astype(np.int64), w.astype(np.float32))

    caps, slots = _plan_slots(per_expert)
    tiles, total = _tiles_for(caps)

    # Per-expert weight layouts (bf16, m-tile-major); computed once per expert.
    experts_used = sorted({s[0] for core in slots for s in core})
    wg_l = {e: _wgu_layout(wg[e]) for e in experts_used}
    wu_l = {e: _wgu_layout(wu[e]) for e in experts_used}
    wd_l = {e: _wd_layout(wd[e]) for e in experts_used}

    groups = [g for g in range(len(caps)) if caps[g] > 0]
    in_maps = []
    for c in range(NCORES):
        xt_flat = np.zeros((128, KD * total), dtype=_BF16)
        for tg_, ts, w, off in tiles:
            idx = slots[c][tg_][1]
            seg = idx[ts : ts + w]
            gx = np.zeros((w, D), dtype=np.float32)
            gx[: len(seg)] = x[seg]
            blk = gx.T.reshape(KD, 128, w).transpose(1, 0, 2).reshape(128, KD * w)
            xt_flat[:, KD * off : KD * (off + w)] = blk.astype(_BF16)
        m = {"xt": xt_flat}
        for g in groups:
            e = slots[c][g][0]
            m[f"w{g}g"] = wg_l[e]
            m[f"w{g}u"] = wu_l[e]
            m[f"w{g}d"] = wd_l[e]
        in_maps.append(m)

    if caps not in _COMPILED:
        _COMPILED[caps] = _build_graph(caps)
    nc = _COMPILED[caps]

    # If the environment lacks antenv.axon_hooks, running with BASS_TRACE=1
    # would crash inside run_bass_kernel_spmd on an unguarded import; provide
    # an inert hook registry so tracing degrades to a warning instead.
    try:
        import antenv.axon_hooks  # noqa: F401
    except Exception:
        import sys as _sys
        import types as _types

        _m = _types.ModuleType("antenv.axon_hooks")
        _m._h = None
        _m.set_axon_ntff_profile_hook = lambda h: setattr(_m, "_h", h)
        _m.get_axon_ntff_profile_hook = lambda: getattr(_m, "_h", None)
        _sys.modules["antenv.axon_hooks"] = _m

    from concourse.bass_utils import run_bass_kernel_spmd

    res = run_bass_kernel_spmd(nc, in_maps, core_ids=list(range(NCORES)))
    global LAST_EXEC_NS, LAST_RESULT
    LAST_EXEC_NS = res.exec_time_ns
    LAST_RESULT = res

    out = np.zeros((T, D), dtype=np.float32)
    for c in range(NCORES):
        yT = res.results[c]["out"]  # [128, KD*total] f32
        for tg_, ts, w, off in tiles:
            e, idx, wt = slots[c][tg_]
            seg = idx[ts : ts + w]
            wseg = wt[ts : ts + w]
            if len(seg) == 0:
                continue
            y = (
                yT[:, KD * off : KD * (off + w)]
                .astype(np.float32)
                .reshape(128, KD, w)
                .transpose(1, 0, 2)
                .reshape(D, w)
                .T
            )
            out[seg] += wseg[:, None] * y[: len(seg)]

    return out.reshape(Bb, S, D)



# revision 30
# speedup vs baseline: 1.0299x; 1.0299x over previous
"""Always-on MoE forward (expert 0 dense + top-k of 7 routed) on 8 TRN2 cores.

Strategy
--------
The router (4096x1024 @ 1024x7 matmul + softmax + top-2) is ~58 MFLOP --
negligible -- so it runs on host in numpy as part of computing the sharding
plan.  The expensive part (expert SwiGLU MLPs, ~155 GFLOP with top-2
sparsity) runs on device, expert-parallel with host-side token
dispatch/combine:

- SPMD graph: every core runs TWO or THREE weight groups with compile-time
  capacities.  Each (core, group) slot is filled with tokens of ONE expert
  (weights supplied per-core via in_maps).  A capacity search (2-group
  legacy + 3-group DFS assignment where an expert may span several slots on
  different cores) picks the group sizes minimizing the modeled PE-stream
  time; with 3 groups the per-core capacity lands within ~1% of the ideal
  (T + k*T) / 8 balance point (e.g. caps (344, 412, 792) -> 1548 vs 1536
  ideal, vs 1586 for the best 2-group plan).  Leftover slots are filled
  with always-on expert-0 tokens.
- Host gathers each core's tokens (transposed k-chunk layout, bf16), device
  computes down(silu(x@wg) * (x@wu)) for both groups, host scatter-adds the
  outputs with the combine weights (expert-0 weight 1.0).

Device kernel: pure dense matmul streaming, weights stationary in SBUF per
group.  All DRAM views are pre-tiled on host into [128, flat] layouts so
every DMA is contiguous per partition and every matmul operand is a direct
SBUF slice; weights are DMA'd per 128-wide m-tile (0.25 MB) so the first
matmuls start ~8 us into the kernel instead of waiting for full weight sets.
A 38-matmul PE pre-warm burst accumulates the ~3.4 us of sustained activity
the HAM clock gate needs, so the gate flips to 8/8 during the DMA ramp, and
the final tile's last m2 output is split column-wise so its cast+DMA overlap
the closing matmuls (shorter serial tail).
"""

import numpy as np
import ml_dtypes

D = 1024
DFF = 2048
E = 8
NCORES = 8
T = 2 * 2048  # B * S
KD = D // 128    # contraction chunks over D
KF = DFF // 128  # contraction chunks over DFF

_COMPILED = {}

_BF16 = ml_dtypes.bfloat16

LAST_EXEC_NS = None
LAST_RESULT = None


def _route(x, router_w, router_b, top_k):
    """Replicates the reference router in numpy f32: returns (topi, topw)."""
    logits = x.astype(np.float32) @ router_w.astype(np.float32) + router_b.astype(
        np.float32
    )
    m = logits.max(axis=-1, keepdims=True)
    p = np.exp(logits - m)
    p /= p.sum(axis=-1, keepdims=True)
    k = int(top_k)
    topi = np.argpartition(-p, kth=k - 1, axis=-1)[:, :k]  # top-k set (unordered)
    topw = np.take_along_axis(p, topi, axis=-1)
    topw = topw / topw.sum(axis=-1, keepdims=True)
    return topi, topw.astype(np.float32)


def _split_even(idx, w, n):
    """Split (idx, w) into n near-equal chunks."""
    c = len(idx)
    sizes = [(c + n - 1 - i) // n for i in range(n)]
    out, pos = [], 0
    for s in sizes:
        out.append((idx[pos : pos + s], w[pos : pos + s]))
        pos += s
    return out


def _widths_for(cap):
    """Tile widths for a group capacity.

    First tile as wide as possible (512) -- during the first tile the weight
    stream runs near the HBM rate, and a wider tile lowers the per-ns weight
    demand.  Remaining capacity in near-equal tiles, kept above the ~280
    matmul instruction floor (LDWEIGHTS + dispatch) when possible.
    """
    if cap <= 0:
        return []

    def near_equal(c, n):
        return [(c + n - 1 - i) // n for i in range(n)]

    plain = near_equal(cap, -(-cap // 512))
    if cap > 512:
        rem = cap - 512
        lead = [512] + near_equal(rem, -(-rem // 512))
    else:
        lead = plain
    cost = lambda ws: sum(max(w, 280) for w in ws)
    return lead if cost(lead) <= cost(plain) else plain


def _tile_cost(cap):
    """Modeled PE-stream ns for one group: 384 matmuls per tile, each
    max(streaming, LDWEIGHTS-floor ~108ns)."""
    return sum(384 * max(w / 2.4 + 2.5, 108.0) for w in _widths_for(cap))


def _assign_k3(caps, counts_list):
    """DFS: assign each routed expert a (i0,i1,i2) slot combo (i_g slots of
    group g); e0 must fit in the leftover slots.  Cost is fixed by `caps`,
    so the FIRST feasible assignment wins (min-capacity-first ordering keeps
    leftover for e0 large).  Returns {expert: (i0,i1,i2)} or None."""
    sizes = caps
    total_cap = NCORES * sum(sizes)
    order = sorted(counts_list, key=lambda ec: -ec[1])
    combos_per = []
    for e, c in order:
        opts = []
        for i0 in range(4):
            for i1 in range(4):
                for i2 in range(4):
                    capx = i0 * sizes[0] + i1 * sizes[1] + i2 * sizes[2]
                    if capx >= c:
                        opts.append((capx, (i0, i1, i2)))
        opts.sort()
        if not opts:
            return None
        combos_per.append((e, c, opts[:10]))
    # lower bound on remaining assigned capacity from expert idx onward
    lb = [0] * (len(combos_per) + 1)
    for i in range(len(combos_per) - 1, -1, -1):
        lb[i] = lb[i + 1] + combos_per[i][2][0][0]
    found = [None]
    budget = [60000]
    def dfs(idx, use, tot, picks):
        if found[0] is not None or budget[0] <= 0:
            return
        budget[0] -= 1
        if total_cap - tot - lb[idx] < T:
            return  # e0 can no longer fit
        if idx == len(combos_per):
            found[0] = dict(picks)
            return
        e, c, opts = combos_per[idx]
        for capx, inc in opts:
            nu = [u + i for u, i in zip(use, inc)]
            if any(u > NCORES for u in nu):
                continue
            picks.append((e, inc))
            dfs(idx + 1, nu, tot + capx, picks)
            picks.pop()
            if found[0] is not None:
                return
    dfs(0, [0, 0, 0], 0, [])
    return found[0]


def _plan_k2(counts):
    """Legacy 2-group plan: (cost, caps, combos) -- each routed expert gets
    j B-slots; e0 fills the rest."""
    cands = sorted(
        {-(-c // j) for c in counts.values() for j in range(1, 9) if c} | {512}
    )
    best = None
    for Bc in cands:
        npieces = sum(-(-c // Bc) for c in counts.values() if c)
        if npieces > NCORES:
            continue
        nfree = NCORES - npieces
        e0_in_b = min(T, nfree * Bc)
        A = -(-(T - e0_in_b) // NCORES) if e0_in_b < T else 0
        tot = _tile_cost(A) + _tile_cost(Bc)
        if best is None or tot < best[0]:
            best = (tot, A, Bc)
    tot, A, B = best
    combos = {e: (0, -(-c // B)) for e, c in counts.items() if c}
    return tot, (A, B), combos


def _plan_k3(counts):
    """Search 3-group capacities; returns (cost, caps, combos) or None."""
    from functools import lru_cache

    tc = lru_cache(maxsize=None)(_tile_cost)
    lo = -(-(T + sum(counts.values())) // NCORES)
    counts_list = [(e, c) for e, c in counts.items() if c > 0]
    cands = []
    for C in range(560, 1241, 4):
        cC = tc(C)
        for B in range(240, C + 1, 4):
            cB = tc(B)
            a0 = max(160, lo - B - C)
            a1 = min(B, lo + 52 - B - C)
            a0 += (-a0) % 4
            for A in range(a0, a1 + 1, 4):
                cands.append((tc(A) + cB + cC, A, B, C))
    cands.sort()
    tried = 0
    for cost, A, B, C in cands:
        tried += 1
        if tried > 25000:
            break
        picks = _assign_k3((A, B, C), counts_list)
        if picks is not None:
            return cost, (A, B, C), picks
    return None


def _plan_slots(per_expert):
    """Pick group capacities and fill the 8 slots per group.

    per_expert: {e: (idx, w)} for routed experts.
    Returns (caps, slots); slots[c] = tuple over groups of (e, idx, w).
    """
    counts = {e: len(v[0]) for e, v in per_expert.items()}
    cost2, caps2, combos2 = _plan_k2(counts)
    plan3 = _plan_k3(counts)
    if plan3 is not None and plan3[0] < cost2 - 2000:
        cost, caps, combos = plan3
    else:
        cost, caps, combos = cost2, caps2, combos2

    ngroups = len(caps)
    ones = np.ones(T, dtype=np.float32)
    e0_idx = np.arange(T, dtype=np.int64)

    # Build expert pieces per group: fill assigned slots, remainder last.
    group_pieces = [[] for _ in range(ngroups)]
    for e in sorted(counts, key=lambda e: -counts[e]):
        if counts[e] == 0:
            continue
        idx, w = per_expert[e]
        combo = combos[e]
        slot_sizes = []
        for g in range(ngroups):
            slot_sizes += [(g, caps[g])] * combo[g]
        # fill largest slots first so at most one slot is partial
        slot_sizes.sort(key=lambda gs: -gs[1])
        pos = 0
        for g, size in slot_sizes:
            take = min(size, len(idx) - pos)
            group_pieces[g].append((e, idx[pos : pos + take], w[pos : pos + take]))
            pos += take
        assert pos >= len(idx), f"expert {e} tokens not covered"
    # e0 fills remaining slots (largest first so at most one is partial)
    free = []
    for g in range(ngroups):
        free += [(g, caps[g])] * (NCORES - len(group_pieces[g]))
    free.sort(key=lambda gs: -gs[1])
    pos = 0
    for g, size in free:
        take = min(size, T - pos)
        group_pieces[g].append((0, e0_idx[pos : pos + take], ones[pos : pos + take]))
        pos += take
    assert pos >= T, "expert-0 tokens not covered"

    slots = [
        tuple(group_pieces[g][c] for g in range(ngroups)) for c in range(NCORES)
    ]
    return caps, slots


def _wgu_layout(w2d):
    """[D, DFF] f32 -> [128, KF*KD*128] bf16 m-tile-major layout:
    element [p, (m*KD + k)*128 + c] = W[k*128 + p, m*128 + c]."""
    a = w2d.reshape(KD, 128, KF, 128).transpose(1, 2, 0, 3).reshape(128, -1)
    return np.ascontiguousarray(a).astype(_BF16)


def _wd_layout(w2d):
    """[DFF, D] f32 -> [128, KD*KF*128] bf16 m-tile-major layout:
    element [p, (m*KF + k)*128 + c] = W[k*128 + p, m*128 + c]."""
    a = w2d.reshape(KF, 128, KD, 128).transpose(1, 2, 0, 3).reshape(128, -1)
    return np.ascontiguousarray(a).astype(_BF16)


def _tiles_for(caps):
    """Token tiles [(group, start_in_group, width, flat_offset)] per core."""
    tiles = []
    off = 0
    for g, cap in enumerate(caps):
        ts = 0
        for w in _widths_for(cap):
            tiles.append((g, ts, w, off))
            off += w
            ts += w
    return tiles, off


def _build_graph(caps):
    import concourse.mybir as mybir
    import concourse.tile as tile
    from concourse import bacc
    from contextlib import ExitStack

    bf16 = mybir.dt.bfloat16
    f32 = mybir.dt.float32

    tiles, total = _tiles_for(caps)
    groups = [g for g in range(len(caps)) if caps[g] > 0]

    nc = bacc.Bacc("TRN2", target_bir_lowering=False)

    xt_d = nc.declare_dram_parameter("xt", [128, KD * total], bf16, isOutput=False)
    w_ds = []
    for g in groups:
        w_ds.append(
            (
                nc.declare_dram_parameter(
                    f"w{g}g", [128, KF * KD * 128], bf16, isOutput=False
                ),
                nc.declare_dram_parameter(
                    f"w{g}u", [128, KF * KD * 128], bf16, isOutput=False
                ),
                nc.declare_dram_parameter(
                    f"w{g}d", [128, KD * KF * 128], bf16, isOutput=False
                ),
            )
        )
    out_d = nc.declare_dram_parameter("out", [128, KD * total], bf16, isOutput=True)

    with tile.TileContext(nc) as tc, ExitStack() as ctx:
        # One SBUF pool + one PSUM pool (per-tag bufs) -- the end-of-kernel
        # semaphore drain scales with pool count, so fewer pools shorten the
        # epilogue.
        sbp = ctx.enter_context(tc.tile_pool(name="sb", bufs=1))
        wpool = xpool = hpool = gpool = opool = sbp
        psp = ctx.enter_context(tc.tile_pool(name="ps", bufs=2, space="PSUM"))
        psg = psu = psd = psp

        GU_M = KD * 128  # bytes-per-m-tile span (elems) for wg/wu
        D_M = KF * 128   # for wd

        # PE pre-warm: a short burst of dummy matmuls bridges the ~2us gap
        # until the first weight/token chunks land, and starts the HAM
        # busy-window early so the clock gate flips to 8/8 sooner.  Real
        # matmuls start as soon as DMA delivers (cold at first, but doing
        # real work at 1.2 GHz beats idling warm).
        warm_sb = wpool.tile([128, 128], bf16, tag="warm")
        nc.gpsimd.memset(warm_sb[:], 0)
        ps_w = psp.tile([128, 128], f32, tag="psw", bufs=1)
        for _ in range(38):
            nc.tensor.matmul(ps_w[:], warm_sb[:], warm_sb[:], start=True, stop=True)

        # Larger group first: its weights stream in unblocked at t=0, and the
        # other group's weight reloads (WAR-gated on this group's last use of
        # each m-tile slot) get a long compute window to hide under.
        order = sorted(range(len(groups)), key=lambda gi: -caps[groups[gi]])

        # Weight DMA blocks: the HWDGE issue path allows only ~6 outstanding
        # dma_starts per engine (each further issue waits a prior completion
        # sem), so many small DMAs serialize on sem round-trips.  Geometric
        # doubling gives a tiny first block (first matmul starts early) and
        # few total DMAs.  wg+wd ride sync (wd queued behind wg keeps it off
        # the ramp-critical bytes; SDMA bandwidth is shared, so concurrent
        # wd DMAs starve the wg/wu/xt lead blocks -- measured +23 us), and
        # xt + wu + outs ride scalar, with xt tile0 split into 4 k-chunks
        # interleaved with the wu lead blocks so the first real matmul
        # starts ~0.4 MB into the stream instead of 1.25 MB.
        GU_BLKS = [(0, 1), (1, 2), (2, 4), (4, 8), (8, 16)]   # m-tile ranges
        WD_BLKS = [(0, 2), (2, 4), (4, 8)]                    # m2-tile ranges

        for gi in order:
            g = groups[gi]
            wg_d, wu_d, wd_d = w_ds[gi]
            first_group = gi == order[0]
            wg_blk, wu_blk, wd_blk = [], [], []
            for bi, (s, e) in enumerate(GU_BLKS):
                span = (e - s) * GU_M
                tg = wpool.tile([128, span], bf16, tag=f"wg_b{bi}")
                nc.sync.dma_start(tg[:], wg_d.ap()[:, s * GU_M : e * GU_M])
                wg_blk.append(tg)

            xt0_sb = None
            if first_group:
                t0_off, t0_w = next(
                    (off, w_) for tg2, ts2, w_, off in tiles if tg2 == g
                )
                xt0_sb = xpool.tile([128, KD * t0_w], bf16, tag="xt", bufs=3)

                def xt0_chunk(ci, t=xt0_sb, off=t0_off, w_=t0_w):
                    # chunks 0-1 ride scalar HWDGE (fast first byte); 2-3
                    # ride the otherwise-idle gpsimd SWDGE (~2us latency,
                    # fine for data needed ~5-8us in).
                    eng = nc.scalar if ci < 2 else nc.gpsimd
                    ks, ke = 2 * ci, 2 * ci + 2
                    eng.dma_start(
                        t[:, ks * w_ : ke * w_],
                        xt_d.ap()[:, KD * off + ks * w_ : KD * off + ke * w_],
                    )

                xt0_chunk(0)
                xt0_chunk(2)
                xt0_chunk(3)
            for bi, (s, e) in enumerate(GU_BLKS):
                span = (e - s) * GU_M
                tu = wpool.tile([128, span], bf16, tag=f"wu_b{bi}")
                nc.scalar.dma_start(tu[:], wu_d.ap()[:, s * GU_M : e * GU_M])
                wu_blk.append(tu)
                if first_group and bi == 0:
                    xt0_chunk(1)
            for bi, (s, e) in enumerate(WD_BLKS):
                span = (e - s) * D_M
                td = wpool.tile([128, span], bf16, tag=f"wd_b{bi}")
                nc.sync.dma_start(td[:], wd_d.ap()[:, s * D_M : e * D_M])
                wd_blk.append(td)

            def gu_slice(blk_list, m, k):
                for bi, (s, e) in enumerate(GU_BLKS):
                    if s <= m < e:
                        base = ((m - s) * KD + k) * 128
                        return blk_list[bi][:, base : base + 128]
                raise AssertionError

            def wd_slice(m2, k2):
                for bi, (s, e) in enumerate(WD_BLKS):
                    if s <= m2 < e:
                        base = ((m2 - s) * KF + k2) * 128
                        return wd_blk[bi][:, base : base + 128]
                raise AssertionError

            group_tiles = [t for t in tiles if t[0] == g]
            for ti, (tg_, ts, w, off) in enumerate(group_tiles):
                final_tile = gi == order[-1] and ti == len(group_tiles) - 1
                if first_group and ts == 0:
                    rhs = lambda k, t=xt0_sb: t[:, k * w : k * w + w]
                else:
                    xt_sb = xpool.tile([128, KD * w], bf16, tag="xt", bufs=3)
                    nc.scalar.dma_start(
                        xt_sb[:], xt_d.ap()[:, KD * off : KD * (off + w)]
                    )
                    rhs = lambda k, t=xt_sb: t[:, k * w : k * w + w]
                h_sb = hpool.tile([128, KF, 512], bf16, tag="h", bufs=2)
                for m in range(KF):
                    ps_g = psg.tile([128, 512], f32, tag="psg")
                    ps_u = psu.tile([128, 512], f32, tag="psu")
                    for k in range(KD):
                        nc.tensor.matmul(
                            ps_g[:, :w],
                            gu_slice(wg_blk, m, k),
                            rhs(k),
                            start=(k == 0),
                            stop=(k == KD - 1),
                        )
                    for k in range(KD):
                        nc.tensor.matmul(
                            ps_u[:, :w],
                            gu_slice(wu_blk, m, k),
                            rhs(k),
                            start=(k == 0),
                            stop=(k == KD - 1),
                        )
                    g_sb = gpool.tile([128, 512], bf16, tag="gact", bufs=3)
                    nc.scalar.activation(
                        g_sb[:, :w],
                        ps_g[:, :w],
                        mybir.ActivationFunctionType.Silu,
                    )
                    nc.vector.tensor_mul(h_sb[:, m, :w], g_sb[:, :w], ps_u[:, :w])
                # Output: one DMA per tile (fewer descriptors, shorter
                # end-of-kernel drain), except the final tile which keeps
                # per-m2 DMAs so the last bytes leave as soon as computed.
                o_sb = opool.tile([128, KD * w], bf16, tag="o", bufs=2)
                for m2 in range(KD):
                    if final_tile and m2 == KD - 1:
                        # Split the last m2 column-wise and alternate the
                        # out DMAs across the Act and (idle) sync rings:
                        # the cast+DMA of the first half overlaps the
                        # second half's matmuls, and the very last DMA
                        # issues onto an empty queue.
                        hw1 = (w // 2 + 3) & ~3
                        for ci2, (cs, cw) in enumerate(((0, hw1), (hw1, w - hw1))):
                            ps_d = psd.tile([128, 512], f32, tag="psd")
                            for k2 in range(KF):
                                nc.tensor.matmul(
                                    ps_d[:, :cw],
                                    wd_slice(m2, k2),
                                    h_sb[:, k2, cs : cs + cw],
                                    start=(k2 == 0),
                                    stop=(k2 == KF - 1),
                                )
                            nc.vector.tensor_copy(
                                o_sb[:, m2 * w + cs : m2 * w + cs + cw],
                                ps_d[:, :cw],
                            )
                            eng = nc.scalar if ci2 == 0 else nc.sync
                            eng.dma_start(
                                out_d.ap()[
                                    :,
                                    KD * off + m2 * w + cs : KD * off
                                    + m2 * w
                                    + cs
                                    + cw,
                                ],
                                o_sb[:, m2 * w + cs : m2 * w + cs + cw],
                            )
                        continue
                    ps_d = psd.tile([128, 512], f32, tag="psd")
                    for k2 in range(KF):
                        nc.tensor.matmul(
                            ps_d[:, :w],
                            wd_slice(m2, k2),
                            h_sb[:, k2, :w],
                            start=(k2 == 0),
                            stop=(k2 == KF - 1),
                        )
                    nc.vector.tensor_copy(o_sb[:, m2 * w : (m2 + 1) * w], ps_d[:, :w])
                    if final_tile and m2 == KD - 3:
                        # one batched DMA for m2 0-5 (fewer Act-queue issue
                        # slices ahead of the tail-critical DMAs)
                        nc.scalar.dma_start(
                            out_d.ap()[:, KD * off : KD * off + (KD - 2) * w],
                            o_sb[:, 0 : (KD - 2) * w],
                        )
                    elif final_tile and m2 == KD - 2:
                        nc.sync.dma_start(
                            out_d.ap()[
                                :, KD * off + m2 * w : KD * off + (m2 + 1) * w
                            ],
                            o_sb[:, m2 * w : (m2 + 1) * w],
                        )
                if not final_tile:
                    nc.scalar.dma_start(
                        out_d.ap()[:, KD * off : KD * (off + w)], o_sb[:]
                    )

    nc.compile()
    return nc


def kernel(hidden_states, router_w, router_b, wg, wu, wd, top_k):
    hidden_states = np.asarray(hidden_states, dtype=np.float32)
    router_w = np.asarray(router_w, dtype=np.float32)
    router_b = np.asarray(router_b, dtype=np.float32)
    wg = np.asarray(wg, dtype=np.float32)
    wu = np.asarray(wu, dtype=np.float32)
    wd = np.asarray(wd, dtype=np.float32)

    Bb, S, Dd = hidden_states.shape
    x = hidden_states.reshape(-1, Dd)
    assert x.shape == (T, D)

    topi, topw = _route(x, router_w, router_b, top_k)
    per_expert = {}
    for e in range(1, E):
        sel = np.nonzero((topi == (e - 1)).any(axis=1))[0]
        w = topw[sel][topi[sel] == (e - 1)]
        per_expert[e] = (sel.astype(np.int64), w.astype(np.float32))

    caps, slots = _plan_slots(per_expert)
    tiles, total = _tiles_for(caps)

    # Per-expert weight layouts (bf16, m-tile-major); computed once per expert.
    experts_used = sorted({s[0] for core in slots for s in core})
    wg_l = {e: _wgu_layout(wg[e]) for e in experts_used}
    wu_l = {e: _wgu_layout(wu[e]) for e in experts_used}
    wd_l = {e: _wd_layout(wd[e]) for e in experts_used}

    groups = [g for g in range(len(caps)) if caps[g] > 0]
    in_maps = []
    for c in range(NCORES):
        xt_flat = np.zeros((128, KD * total), dtype=_BF16)
        for tg_, ts, w, off in tiles:
            idx = slots[c][tg_][1]
            seg = idx[ts : ts + w]
            gx = np.zeros((w, D), dtype=np.float32)
            gx[: len(seg)] = x[seg]
            blk = gx.T.reshape(KD, 128, w).transpose(1, 0, 2).reshape(128, KD * w)
            xt_flat[:, KD * off : KD * (off + w)] = blk.astype(_BF16)
        m = {"xt": xt_flat}
        for g in groups:
            e = slots[c][g][0]
            m[f"w{g}g"] = wg_l[e]
            m[f"w{g}u"] = wu_l[e]
            m[f"w{g}d"] = wd_l[e]
        in_maps.append(m)

    if caps not in _COMPILED:
        _COMPILED[caps] = _build_graph(caps)
    nc = _COMPILED[caps]

    # If the environment lacks antenv.axon_hooks, running with BASS_TRACE=1
    # would crash inside run_bass_kernel_spmd on an unguarded import; provide
    # an inert hook registry so tracing degrades to a warning instead.
    try:
        import antenv.axon_hooks  # noqa: F401
    except Exception:
        import sys as _sys
        import types as _types

        _m = _types.ModuleType("antenv.axon_hooks")
        _m._h = None
        _m.set_axon_ntff_profile_hook = lambda h: setattr(_m, "_h", h)
        _m.get_axon_ntff_profile_hook = lambda: getattr(_m, "_h", None)
        _sys.modules["antenv.axon_hooks"] = _m

    from concourse.bass_utils import run_bass_kernel_spmd

    res = run_bass_kernel_spmd(nc, in_maps, core_ids=list(range(NCORES)))
    global LAST_EXEC_NS, LAST_RESULT
    LAST_EXEC_NS = res.exec_time_ns
    LAST_RESULT = res

    out = np.zeros((T, D), dtype=np.float32)
    for c in range(NCORES):
        yT = res.results[c]["out"]  # [128, KD*total] f32
        for tg_, ts, w, off in tiles:
            e, idx, wt = slots[c][tg_]
            seg = idx[ts : ts + w]
            wseg = wt[ts : ts + w]
            if len(seg) == 0:
                continue
            y = (
                yT[:, KD * off : KD * (off + w)]
                .astype(np.float32)
                .reshape(128, KD, w)
                .transpose(1, 0, 2)
                .reshape(D, w)
                .T
            )
            out[seg] += wseg[:, None] * y[: len(seg)]

    return out.reshape(Bb, S, D)

